# revision 2
# baseline (speedup 1.0000x reference)
"""Sparse attention kernel for Trainium2 (8 NeuronCores, data-parallel over batch).

Reference computation (per batch row b):
    q    = x @ q_w.T                                  [N, C]
    xkv  = x[key_ind]                                 [NKV, C]
    kv   = xkv @ kv_w.T -> per-head k, v              [NKV, 2C]
    attn = softmax((q*scale) @ k.T) @ v               [N, C]
    out  = attn @ proj_w.T + proj_b                   [N, C]

Per-core layout (core = one batch row), all SBUF data bf16, PSUM f32:
  - Everything transposed ("feature on partition"): qT [C, N] per head pair,
    kT [pair, NKV], scores ST [keys, tokens] so softmax needs no transposes;
    denominator via an all-ones pre-fill of the augmented v (65th column).
  - One PSUM pool, 8 banks: st 2x[128,1024] (two score tiles share one exp),
    ov 2x[128,512], mm 2x[128,512] (kv/q/proj accumulators share a tag).
  - KV gather via indirect SWDGE DMA + PE transposes (bf16: 1 cyc/row).
  - Few, large DMAs ordered so the gather wins the DMA engines early.
"""
import os
import sys

sys.path.insert(0, "/opt/trn_rl_repo")

import numpy as np  # noqa: E402
import ml_dtypes  # noqa: E402

B, N, C = 8, 2048, 768
NKV = 512
H = 12
HD = C // H          # 64
SCALE = HD ** -0.5
P = 128
CT = C // P          # 6 feature tiles
NC2 = 512            # token chunk
NCH = N // NC2       # 4 chunks
MCH = NKV // P       # 4 key chunks
G = H // 2           # 6 head pairs

_CACHE = {}


def _build():
    import concourse.bass as bass
    import concourse.mybir as mybir
    import concourse.tile as tile
    from concourse import bacc
    from concourse.masks import make_identity
    from contextlib import ExitStack

    F32 = mybir.dt.float32
    BF16 = mybir.dt.bfloat16
    I32 = mybir.dt.int32
    Exp = mybir.ActivationFunctionType.Exp

    nc = bacc.Bacc("TRN2", target_bir_lowering=False, debug=False, num_devices=8)

    xT = nc.dram_tensor("xT", [NCH, P, CT * NC2], BF16, kind="ExternalInput")
    xr = nc.dram_tensor("xr", [N, C], BF16, kind="ExternalInput")
    idx = nc.dram_tensor("idx", [P, MCH], I32, kind="ExternalInput")
    wq = nc.dram_tensor("wq", [P, CT * C], BF16, kind="ExternalInput")
    wk = nc.dram_tensor("wk", [P, CT * C], BF16, kind="ExternalInput")
    wv = nc.dram_tensor("wv", [P, CT * C], BF16, kind="ExternalInput")
    wp = nc.dram_tensor("wp", [P, CT * C], BF16, kind="ExternalInput")
    pb = nc.dram_tensor("pb", [P, CT], F32, kind="ExternalInput")
    out = nc.dram_tensor("out", [P, CT, NCH, NC2], BF16, kind="ExternalOutput")

    with tile.TileContext(nc) as tc, ExitStack() as top:
        const = top.enter_context(tc.tile_pool(name="const", bufs=1))
        gat = top.enter_context(tc.tile_pool(name="gat", bufs=2))
        qtp = top.enter_context(tc.tile_pool(name="qtp", bufs=2))
        ptp = top.enter_context(tc.tile_pool(name="ptp", bufs=6))
        att = top.enter_context(tc.tile_pool(name="att", bufs=2))
        rcp = top.enter_context(tc.tile_pool(name="rcp", bufs=4))
        ojp = top.enter_context(tc.tile_pool(name="ojp", bufs=3))
        ps = top.enter_context(tc.tile_pool(name="ps", bufs=1, space="PSUM"))

        # ---------- input DMAs (SP queue, loads only; order matters) ----------
        idx_sb = const.tile([P, MCH], I32, tag="idx")
        nc.sync.dma_start(idx_sb[:], idx[:])
        pb_sb = const.tile([P, CT], F32, tag="pb")
        nc.sync.dma_start(pb_sb[:], pb[:])

        ident = const.tile([P, P], BF16, tag="ident")
        make_identity(nc, ident[:])

        # gathers (Pool queue; only DMA-engine users until wq's transfer)
        xkv_tiles = []
        for k in range(MCH):
            xkv = gat.tile([P, C], BF16, tag="xkv")
            nc.gpsimd.indirect_dma_start(
                out=xkv[:], out_offset=None, in_=xr[:],
                in_offset=bass.IndirectOffsetOnAxis(ap=idx_sb[:, k:k + 1], axis=0))
            xkv_tiles.append(xkv)

        wk_sb = const.tile([P, CT * C], BF16, tag="wk")
        nc.sync.dma_start(wk_sb[:], wk[:])
        wv_sb = const.tile([P, CT * C], BF16, tag="wv")
        nc.sync.dma_start(wv_sb[:], wv[:])
        wq_sb = const.tile([P, CT * C], BF16, tag="wq")
        nc.sync.dma_start(wq_sb[:], wq[:])
        xTc_sb = []
        for ch in range(NCH):
            t = const.tile([P, CT * NC2], BF16, tag=f"xT{ch}")
            nc.sync.dma_start(t[:], xT[ch, :, :])
            xTc_sb.append(t)
        wp_sb = const.tile([P, CT * C], BF16, tag="wp")
        nc.sync.dma_start(wp_sb[:], wp[:])

        # ---------- KV phase: gather transpose + k/v projections ----------
        xkvT = const.tile([P, CT * NKV], BF16, tag="xkvT")
        xkvT3 = xkvT[:].rearrange("p (i m) -> p i m", i=CT)
        for k in range(MCH):
            tr = ps.tile([P, CT * P], BF16, tag="ov", bufs=2)
            for i in range(CT):
                nc.tensor.transpose(tr[:, i * P:(i + 1) * P],
                                    xkv_tiles[k][:, i * P:(i + 1) * P], ident[:])
            nc.vector.tensor_copy(xkvT3[:, :, k * P:(k + 1) * P],
                                  tr[:].rearrange("p (i m) -> p i m", i=CT))

        # kT: per head pair g -> [128, NKV] (rows 0-63 head 2g, 64-127 head 2g+1)
        kT_sb = const.tile([P, G * NKV], BF16, tag="kT")
        for g in range(G):
            kp = ps.tile([P, NKV], F32, tag="mm", bufs=2)
            for i in range(CT):
                nc.tensor.matmul(kp[:], wk_sb[:, i * C + g * P:i * C + (g + 1) * P],
                                 xkvT3[:, i, :], start=(i == 0), stop=(i == CT - 1))
            nc.vector.tensor_copy(kT_sb[:, g * NKV:(g + 1) * NKV], kp[:])

        # v (+ ones col): per key chunk [128 keys, H*(HD+1)], col HD of each
        # head block stays 1.0 from the memset pre-fill
        vaug_sb = []
        for k in range(MCH):
            va = const.tile([P, H * (HD + 1)], BF16, tag=f"vaug{k}")
            nc.gpsimd.memset(va[:], 1.0)
            va3 = va[:].rearrange("p (h x) -> p h x", x=HD + 1)
            for half in range(2):
                vp = ps.tile([P, CT * HD], F32, tag="mm", bufs=2)
                for i in range(CT):
                    nc.tensor.matmul(vp[:], xkvT3[:, i, k * P:(k + 1) * P],
                                     wv_sb[:, i * C + half * CT * HD:
                                           i * C + (half + 1) * CT * HD],
                                     start=(i == 0), stop=(i == CT - 1))
                nc.vector.tensor_copy(va3[:, CT * half:CT * half + CT, 0:HD],
                                      vp[:].rearrange("p (h x) -> p h x", x=HD))
            vaug_sb.append(va)

        # ---------- main loop over token chunks ----------
        for ch in range(NCH):
            # qT per head pair g: [128, NC2]
            qT = []
            for g in range(G):
                qp = ps.tile([P, NC2], F32, tag="mm", bufs=2)
                for i in range(CT):
                    nc.tensor.matmul(qp[:], wq_sb[:, i * C + g * P:i * C + (g + 1) * P],
                                     xTc_sb[ch][:, i * NC2:(i + 1) * NC2],
                                     start=(i == 0), stop=(i == CT - 1))
                qt = qtp.tile([P, NC2], BF16, tag=f"qT{g}")
                nc.vector.tensor_copy(qt[:], qp[:])
                qT.append(qt)

            # attention per head pair; attn[g] [128, NC2] rows 0-63 head 2g,
            # rows 64-127 head 2g+1
            attn = []
            for g in range(G):
                at = att.tile([P, NC2], BF16, tag=f"attn{g}")
                for par in range(2):
                    h = 2 * g + par
                    base = par * HD
                    pts = []
                    for half in range(2):
                        st2 = ps.tile([P, 2 * NC2], F32, tag="st", bufs=2)
                        for hh in range(2):
                            k = 2 * half + hh
                            nc.tensor.matmul(
                                st2[:, hh * NC2:(hh + 1) * NC2],
                                kT_sb[base:base + HD,
                                      g * NKV + k * P:g * NKV + (k + 1) * P],
                                qT[g][base:base + HD, :],
                                start=True, stop=True)
                        pt = ptp.tile([P, 2 * NC2], BF16, tag="pt")
                        nc.scalar.activation(pt[:], st2[:], Exp, scale=SCALE)
                        pts.append(pt)
                    ov = ps.tile([HD + 1, NC2], F32, tag="ov", bufs=2)
                    for half in range(2):
                        for hh in range(2):
                            k = 2 * half + hh
                            nc.tensor.matmul(
                                ov[:], vaug_sb[k][:, h * (HD + 1):(h + 1) * (HD + 1)],
                                pts[half][:, hh * NC2:(hh + 1) * NC2],
                                start=(k == 0), stop=(k == MCH - 1))
                    rc = rcp.tile([1, NC2], F32, tag="rc")
                    nc.vector.reciprocal(rc[:], ov[HD:HD + 1, :])
                    rb = rcp.tile([HD, NC2], F32, tag="rb")
                    nc.gpsimd.partition_broadcast(rb[:], rc[:])
                    nc.vector.tensor_mul(at[base:base + HD, :], ov[0:HD, :], rb[:])
                attn.append(at)

            # output projection + bias
            for j in range(CT):
                pp = ps.tile([P, NC2], F32, tag="mm", bufs=2)
                for i in range(CT):
                    nc.tensor.matmul(
                        pp[:], wp_sb[:, i * C + j * P:i * C + (j + 1) * P],
                        attn[i][:], start=(i == 0), stop=(i == CT - 1))
                oj = ojp.tile([P, NC2], BF16, tag="oj")
                nc.vector.tensor_scalar_add(oj[:], pp[:], pb_sb[:, j:j + 1])
                nc.sync.dma_start(out[:, j, ch, :], oj[:])

    nc.compile()
    return nc


def _get_nc():
    if "nc" not in _CACHE:
        _CACHE["nc"] = _build()
    return _CACHE["nc"]


def _prep_core_inputs(x, key_ind, q_w, kv_w, proj_w, proj_b):
    """Build the 8 per-core input maps (everything bf16 except idx/pb)."""
    bf16 = ml_dtypes.bfloat16

    def wT_pack(w):
        # [C(out), C(in)] weight -> transposed blocks [P, CT*C] bf16
        return np.ascontiguousarray(
            w.T.astype(np.float32).reshape(CT, P, C).transpose(1, 0, 2)
            .reshape(P, CT * C)).astype(bf16)

    wq = wT_pack(q_w)
    kvwT3 = kv_w.T.astype(np.float32).reshape(C, H, 2 * HD)
    wk = wT_pack(np.ascontiguousarray(kvwT3[:, :, :HD].reshape(C, C)).T)
    wv = wT_pack(np.ascontiguousarray(kvwT3[:, :, HD:].reshape(C, C)).T)
    wp = wT_pack(proj_w)
    pbp = np.ascontiguousarray(proj_b.astype(np.float32).reshape(CT, P).T)
    x = np.asarray(x, dtype=np.float32)
    in_maps = []
    for b in range(B):
        xb = x[b]                                   # [N, C]
        xTb = np.ascontiguousarray(
            xb.T.reshape(CT, P, NCH, NC2).transpose(2, 1, 0, 3)
            .reshape(NCH, P, CT * NC2)).astype(bf16)
        idxb = np.ascontiguousarray(
            np.asarray(key_ind[b]).astype(np.int32).reshape(MCH, P).T)
        in_maps.append({
            "xT": xTb, "xr": xb.astype(bf16), "idx": idxb,
            "wq": wq, "wk": wk, "wv": wv, "wp": wp, "pb": pbp,
        })
    return in_maps


def kernel(x, key_ind, q_w, kv_w, proj_w, proj_b, _trace=False, _results=None):
    from concourse.bass_utils import run_bass_kernel_spmd

    nc = _get_nc()
    in_maps = _prep_core_inputs(x, key_ind, q_w, kv_w, proj_w, proj_b)
    res = run_bass_kernel_spmd(nc, in_maps, core_ids=list(range(B)), trace=_trace)
    if _results is not None:
        _results.append(res)
    outp = np.empty((B, N, C), dtype=np.float32)
    for b in range(B):
        o = res.results[b]["out"].astype(np.float32)   # [P, CT, NCH, NC2]
        outp[b] = o.transpose(2, 3, 1, 0).reshape(N, C)
    return outp


# revision 6
# speedup vs baseline: 1.1666x; 1.1666x over previous
"""Sparse attention kernel for Trainium2 (8 NeuronCores, data-parallel over batch).

Reference computation (per batch row b):
    q    = x @ q_w.T                                  [N, C]
    xkv  = x[key_ind]                                 [NKV, C]
    kv   = xkv @ kv_w.T -> per-head k, v              [NKV, 2C]
    attn = softmax((q*scale) @ k.T) @ v               [N, C]
    out  = attn @ proj_w.T + proj_b                   [N, C]

Per-core layout (core = one batch row), all SBUF data bf16, PSUM f32:
  - Everything transposed ("feature on partition"): qT [C, N] per head pair,
    kT [pair, NKV], scores ST [keys, tokens] so softmax needs no transposes;
    denominator via an all-ones pre-fill of the augmented v (65th column).
  - One PSUM pool, 8 banks: st 2x[128,1024] (two score tiles share one exp),
    ov 2x[128,512], mm 2x[128,512] (kv/q/proj accumulators share a tag).
  - KV gather via indirect SWDGE DMA + PE transposes (bf16: 1 cyc/row).
  - Engine balance: exp on Act, kv-phase evacs on Act, qt/recip/at-mul on
    DVE, broadcast + bias-add on Pool, loads/stores on SP — PE never waits.
"""
import os
import sys

sys.path.insert(0, "/opt/trn_rl_repo")

import numpy as np  # noqa: E402
import ml_dtypes  # noqa: E402

B, N, C = 8, 2048, 768
NKV = 512
H = 12
HD = C // H          # 64
SCALE = HD ** -0.5
P = 128
CT = C // P          # 6 feature tiles
NC2 = 512            # token chunk
NCH = N // NC2       # 4 chunks
MCH = NKV // P       # 4 key chunks
G = H // 2           # 6 head pairs

_CACHE = {}


def _build():
    import concourse.bass as bass
    import concourse.mybir as mybir
    import concourse.tile as tile
    from concourse import bacc
    from concourse.masks import make_identity
    from contextlib import ExitStack

    F32 = mybir.dt.float32
    BF16 = mybir.dt.bfloat16
    I32 = mybir.dt.int32
    Exp = mybir.ActivationFunctionType.Exp
    Copy = mybir.ActivationFunctionType.Copy

    nc = bacc.Bacc("TRN2", target_bir_lowering=False, debug=False, num_devices=8)

    xT = nc.dram_tensor("xT", [NCH, P, CT * NC2], BF16, kind="ExternalInput")
    xr = nc.dram_tensor("xr", [N, C], BF16, kind="ExternalInput")
    idx = nc.dram_tensor("idx", [P, MCH], I32, kind="ExternalInput")
    wq = nc.dram_tensor("wq", [P, CT * C], BF16, kind="ExternalInput")
    wk = nc.dram_tensor("wk", [P, CT * C], BF16, kind="ExternalInput")
    wv = nc.dram_tensor("wv", [P, CT * C], BF16, kind="ExternalInput")
    wp = nc.dram_tensor("wp", [P, CT * C], BF16, kind="ExternalInput")
    pb = nc.dram_tensor("pb", [P, CT], F32, kind="ExternalInput")
    out = nc.dram_tensor("out", [P, CT, NCH, NC2], BF16, kind="ExternalOutput")

    with tile.TileContext(nc) as tc, ExitStack() as top:
        const = top.enter_context(tc.tile_pool(name="const", bufs=1))
        qtp = top.enter_context(tc.tile_pool(name="qtp", bufs=2))
        ptp = top.enter_context(tc.tile_pool(name="ptp", bufs=6))
        att = top.enter_context(tc.tile_pool(name="att", bufs=2))
        rcp = top.enter_context(tc.tile_pool(name="rcp", bufs=4))
        ojp = top.enter_context(tc.tile_pool(name="ojp", bufs=3))
        ps = top.enter_context(tc.tile_pool(name="ps", bufs=1, space="PSUM"))

        # ---------- input DMAs; gathers get the DMA engines first ----------
        idx_sb = const.tile([P, MCH], I32, tag="idx")
        nc.sync.dma_start(idx_sb[:], idx[:])
        pb_sb = const.tile([P, CT], F32, tag="pb")
        nc.sync.dma_start(pb_sb[:], pb[:])

        ident = const.tile([P, P], BF16, tag="ident")
        make_identity(nc, ident[:])

        xkv_tiles = []
        for k in range(MCH):
            xkv = const.tile([P, C], BF16, tag=f"xkv{k}")
            nc.gpsimd.indirect_dma_start(
                out=xkv[:], out_offset=None, in_=xr[:],
                in_offset=bass.IndirectOffsetOnAxis(ap=idx_sb[:, k:k + 1], axis=0))
            xkv_tiles.append(xkv)

        wq_sb = const.tile([P, CT * C], BF16, tag="wq")
        nc.sync.dma_start(wq_sb[:], wq[:])
        xTc_sb = []
        for ch in range(NCH):
            xTc_sb.append(const.tile([P, CT * NC2], BF16, tag=f"xT{ch}",
                                     name=f"xTc{ch}"))
        nc.sync.dma_start(xTc_sb[0][:], xT[0, :, :])
        wk_sb = const.tile([P, CT * C], BF16, tag="wk")
        nc.sync.dma_start(wk_sb[:], wk[:])
        wv_sb = const.tile([P, CT * C], BF16, tag="wv")
        nc.sync.dma_start(wv_sb[:], wv[:])
        for ch in range(1, NCH):
            nc.sync.dma_start(xTc_sb[ch][:], xT[ch, :, :])
        wp_sb = const.tile([P, CT * C], BF16, tag="wp")
        nc.sync.dma_start(wp_sb[:], wp[:])

        # ---------- KV phase: gather transpose + q(ch0) + k/v projections ----
        xkvT = const.tile([P, CT * NKV], BF16, tag="xkvT")
        xkvT3 = xkvT[:].rearrange("p (i m) -> p i m", i=CT)
        for k in range(MCH):
            tr = ps.tile([P, CT * P], BF16, tag="ov", bufs=2)
            for i in range(CT):
                nc.tensor.transpose(tr[:, i * P:(i + 1) * P],
                                    xkv_tiles[k][:, i * P:(i + 1) * P], ident[:])
            nc.scalar.activation(xkvT3[:, :, k * P:(k + 1) * P],
                                 tr[:].rearrange("p (i m) -> p i m", i=CT), Copy)

        def emit_qp(ch, g):
            qp = ps.tile([P, NC2], F32, tag="mm", bufs=2, name=f"qp{ch}_{g}")
            for i in range(CT):
                nc.tensor.matmul(qp[:], wq_sb[:, i * C + g * P:i * C + (g + 1) * P],
                                 xTc_sb[ch][:, i * NC2:(i + 1) * NC2],
                                 start=(i == 0), stop=(i == CT - 1))
            qt = qtp.tile([P, NC2], BF16, tag=f"qT{g}", name=f"qt{ch}_{g}")
            nc.vector.tensor_copy(qt[:], qp[:])
            return qt

        # q for chunk 0 (fills PE while wk/wv transfers land)
        qT = [emit_qp(0, g) for g in range(G)]

        # kT: per head pair g -> [128, NKV] (rows 0-63 head 2g, 64-127 head 2g+1)
        kT_sb = const.tile([P, G * NKV], BF16, tag="kT")
        for g in range(G):
            kp = ps.tile([P, NKV], F32, tag="mm", bufs=2)
            for i in range(CT):
                nc.tensor.matmul(kp[:], wk_sb[:, i * C + g * P:i * C + (g + 1) * P],
                                 xkvT3[:, i, :], start=(i == 0), stop=(i == CT - 1))
            nc.scalar.activation(kT_sb[:, g * NKV:(g + 1) * NKV], kp[:], Copy)

        # v (+ ones col): per key chunk [128 keys, H*(HD+1)], col HD of each
        # head block stays 1.0 from the memset pre-fill
        vaug_sb = []
        for k in range(MCH):
            va = const.tile([P, H * (HD + 1)], BF16, tag=f"vaug{k}")
            nc.gpsimd.memset(va[:], 1.0)
            va3 = va[:].rearrange("p (h x) -> p h x", x=HD + 1)
            for half in range(2):
                vp = ps.tile([P, CT * HD], F32, tag="mm", bufs=2)
                for i in range(CT):
                    nc.tensor.matmul(vp[:], xkvT3[:, i, k * P:(k + 1) * P],
                                     wv_sb[:, i * C + half * CT * HD:
                                           i * C + (half + 1) * CT * HD],
                                     start=(i == 0), stop=(i == CT - 1))
                nc.scalar.activation(va3[:, CT * half:CT * half + CT, 0:HD],
                                     vp[:].rearrange("p (h x) -> p h x", x=HD), Copy)
            vaug_sb.append(va)

        # ---------- main loop over token chunks ----------
        for ch in range(NCH):
            attn = []
            qT_next = []
            for g in range(G):
                at = att.tile([P, NC2], BF16, tag=f"attn{g}")
                for par in range(2):
                    h = 2 * g + par
                    base = par * HD
                    pts = []
                    for half in range(2):
                        st2 = ps.tile([P, 2 * NC2], F32, tag="st", bufs=2)
                        for hh in range(2):
                            k = 2 * half + hh
                            nc.tensor.matmul(
                                st2[:, hh * NC2:(hh + 1) * NC2],
                                kT_sb[base:base + HD,
                                      g * NKV + k * P:g * NKV + (k + 1) * P],
                                qT[g][base:base + HD, :],
                                start=True, stop=True)
                        pt = ptp.tile([P, 2 * NC2], BF16, tag="pt")
                        nc.scalar.activation(pt[:], st2[:], Exp, scale=SCALE)
                        pts.append(pt)
                    # overlap next chunk's q projection with the exp latency
                    if par == 0 and ch + 1 < NCH:
                        qT_next.append(emit_qp(ch + 1, g))
                    ov = ps.tile([HD + 1, NC2], F32, tag="ov", bufs=2)
                    for half in range(2):
                        for hh in range(2):
                            k = 2 * half + hh
                            nc.tensor.matmul(
                                ov[:], vaug_sb[k][:, h * (HD + 1):(h + 1) * (HD + 1)],
                                pts[half][:, hh * NC2:(hh + 1) * NC2],
                                start=(k == 0), stop=(k == MCH - 1))
                    rc = rcp.tile([1, NC2], F32, tag="rc")
                    nc.vector.reciprocal(rc[:], ov[HD:HD + 1, :])
                    rb = rcp.tile([HD, NC2], F32, tag="rb")
                    nc.gpsimd.partition_broadcast(rb[:], rc[:])
                    nc.vector.tensor_mul(at[base:base + HD, :], ov[0:HD, :], rb[:])
                attn.append(at)

            # output projection + bias (bias-add on Pool, store on SP)
            for j in range(CT):
                pp = ps.tile([P, NC2], F32, tag="mm", bufs=2)
                for i in range(CT):
                    nc.tensor.matmul(
                        pp[:], wp_sb[:, i * C + j * P:i * C + (j + 1) * P],
                        attn[i][:], start=(i == 0), stop=(i == CT - 1))
                oj = ojp.tile([P, NC2], BF16, tag="oj")
                nc.scalar.activation(oj[:], pp[:],
                                     mybir.ActivationFunctionType.Identity,
                                     bias=pb_sb[:, j:j + 1])
                nc.sync.dma_start(out[:, j, ch, :], oj[:])
            qT = qT_next

    nc.compile()
    return nc


def _get_nc():
    if "nc" not in _CACHE:
        _CACHE["nc"] = _build()
    return _CACHE["nc"]


def _prep_core_inputs(x, key_ind, q_w, kv_w, proj_w, proj_b):
    """Build the 8 per-core input maps (everything bf16 except idx/pb)."""
    bf16 = ml_dtypes.bfloat16

    def wT_pack(w):
        # [C(out), C(in)] weight -> transposed blocks [P, CT*C] bf16
        return np.ascontiguousarray(
            w.T.astype(np.float32).reshape(CT, P, C).transpose(1, 0, 2)
            .reshape(P, CT * C)).astype(bf16)

    wq = wT_pack(q_w)
    kvwT3 = kv_w.T.astype(np.float32).reshape(C, H, 2 * HD)
    wk = wT_pack(np.ascontiguousarray(kvwT3[:, :, :HD].reshape(C, C)).T)
    wv = wT_pack(np.ascontiguousarray(kvwT3[:, :, HD:].reshape(C, C)).T)
    wp = wT_pack(proj_w)
    pbp = np.ascontiguousarray(proj_b.astype(np.float32).reshape(CT, P).T)
    x = np.asarray(x, dtype=np.float32)
    in_maps = []
    for b in range(B):
        xb = x[b]                                   # [N, C]
        xTb = np.ascontiguousarray(
            xb.T.reshape(CT, P, NCH, NC2).transpose(2, 1, 0, 3)
            .reshape(NCH, P, CT * NC2)).astype(bf16)
        idxb = np.ascontiguousarray(
            np.asarray(key_ind[b]).astype(np.int32).reshape(MCH, P).T)
        in_maps.append({
            "xT": xTb, "xr": xb.astype(bf16), "idx": idxb,
            "wq": wq, "wk": wk, "wv": wv, "wp": wp, "pb": pbp,
        })
    return in_maps


def kernel(x, key_ind, q_w, kv_w, proj_w, proj_b, _trace=False, _results=None):
    from concourse.bass_utils import run_bass_kernel_spmd

    nc = _get_nc()
    in_maps = _prep_core_inputs(x, key_ind, q_w, kv_w, proj_w, proj_b)
    res = run_bass_kernel_spmd(nc, in_maps, core_ids=list(range(B)), trace=_trace)
    if _results is not None:
        _results.append(res)
    outp = np.empty((B, N, C), dtype=np.float32)
    for b in range(B):
        o = res.results[b]["out"].astype(np.float32)   # [P, CT, NCH, NC2]
        outp[b] = o.transpose(2, 3, 1, 0).reshape(N, C)
    return outp


# revision 12
# speedup vs baseline: 1.3120x; 1.1247x over previous
"""Sparse attention kernel for Trainium2 (8 NeuronCores, data-parallel over batch).

Reference computation (per batch row b):
    q    = x @ q_w.T                                  [N, C]
    xkv  = x[key_ind]                                 [NKV, C]
    kv   = xkv @ kv_w.T -> per-head k, v              [NKV, 2C]
    attn = softmax((q*scale) @ k.T) @ v               [N, C]
    out  = attn @ proj_w.T + proj_b                   [N, C]

Per-core layout (core = one batch row), all SBUF data bf16, PSUM f32:
  - Everything transposed ("feature on partition"): qT [C, N] per head pair,
    kT [pair, NKV], scores ST [keys, tokens] so softmax needs no transposes;
    denominator via an all-ones pre-fill of the augmented v (65th column).
  - One PSUM pool, 8 banks: st 2x[128,1024] (two score tiles share one exp),
    ov 2x[128,512], mm 2x[128,512] (kv/q/proj accumulators share a tag).
  - KV gather via indirect SWDGE DMA + PE transposes (bf16: 1 cyc/row).
  - Engine balance: exp on Act, kv-phase evacs on Act, qt/recip/at-mul on
    DVE, broadcast + bias-add on Pool, loads/stores on SP — PE never waits.
"""
import os
import sys

sys.path.insert(0, "/opt/trn_rl_repo")

import numpy as np  # noqa: E402
import ml_dtypes  # noqa: E402

B, N, C = 8, 2048, 768
NKV = 512
H = 12
HD = C // H          # 64
SCALE = HD ** -0.5
P = 128
CT = C // P          # 6 feature tiles
NC2 = 512            # token chunk
NCH = N // NC2       # 4 chunks
MCH = NKV // P       # 4 key chunks
G = H // 2           # 6 head pairs

_CACHE = {}


def _build():
    import concourse.bass as bass
    import concourse.mybir as mybir
    import concourse.tile as tile
    from concourse import bacc
    from concourse.masks import make_identity
    from contextlib import ExitStack

    F32 = mybir.dt.float32
    BF16 = mybir.dt.bfloat16
    I16 = mybir.dt.int16
    Exp = mybir.ActivationFunctionType.Exp
    Copy = mybir.ActivationFunctionType.Copy

    nc = bacc.Bacc("TRN2", target_bir_lowering=False, debug=False, num_devices=8)

    xT = nc.dram_tensor("xT", [NCH, P, CT * NC2], BF16, kind="ExternalInput")
    xr = nc.dram_tensor("xr", [N, C], BF16, kind="ExternalInput")
    idx = nc.dram_tensor("idx", [P, NKV // 16], I16, kind="ExternalInput")
    wq = nc.dram_tensor("wq", [P, CT * C], BF16, kind="ExternalInput")
    wk = nc.dram_tensor("wk", [P, CT * C], BF16, kind="ExternalInput")
    wv = nc.dram_tensor("wv", [P, CT * C], BF16, kind="ExternalInput")
    wp = nc.dram_tensor("wp", [P, CT * C], BF16, kind="ExternalInput")
    pb = nc.dram_tensor("pb", [P, CT], F32, kind="ExternalInput")
    out = nc.dram_tensor("out", [P, CT, NCH, NC2], BF16, kind="ExternalOutput")

    with tile.TileContext(nc) as tc, ExitStack() as top:
        const = top.enter_context(tc.tile_pool(name="const", bufs=1))
        qtp = top.enter_context(tc.tile_pool(name="qtp", bufs=2))
        ptp = top.enter_context(tc.tile_pool(name="ptp", bufs=6))
        att = top.enter_context(tc.tile_pool(name="att", bufs=2))
        rcp = top.enter_context(tc.tile_pool(name="rcp", bufs=4))
        ojp = top.enter_context(tc.tile_pool(name="ojp", bufs=3))
        ps = top.enter_context(tc.tile_pool(name="ps", bufs=1, space="PSUM"))

        # ---------- input DMAs; the key gather gets the DMA engines early ----
        idx_sb = const.tile([P, NKV // 16], I16, tag="idx")
        nc.sync.dma_start(idx_sb[:], idx[:])
        pb_sb = const.tile([P, CT], F32, tag="pb")
        nc.sync.dma_start(pb_sb[:], pb[:])

        # transposing gather: xkvT3[p, i, j] = xr[key_ind[j], i*128 + p]
        xkvT = const.tile([P, CT * NKV], BF16, tag="xkvT")
        xkvT3 = xkvT[:].rearrange("p (i m) -> p i m", i=CT)
        nc.gpsimd.dma_gather(
            out_ap=xkvT3, in_ap=xr[:], idxs_ap=idx_sb[:], num_idxs=NKV,
            num_idxs_reg=NKV, elem_size=C, transpose=True)

        wq_sb = const.tile([P, CT * C], BF16, tag="wq")
        nc.sync.dma_start(wq_sb[:], wq[:])
        xTc_sb = []
        for ch in range(NCH):
            xTc_sb.append(const.tile([P, CT * NC2], BF16, tag=f"xT{ch}",
                                     name=f"xTc{ch}"))
        nc.sync.dma_start(xTc_sb[0][:], xT[0, :, :])
        wk_sb = const.tile([P, CT * C], BF16, tag="wk")
        nc.sync.dma_start(wk_sb[:], wk[:])
        wv_sb = const.tile([P, CT * C], BF16, tag="wv")
        nc.sync.dma_start(wv_sb[:], wv[:])
        for ch in range(1, NCH):
            nc.sync.dma_start(xTc_sb[ch][:], xT[ch, :, :])
        wp_sb = const.tile([P, CT * C], BF16, tag="wp")
        nc.sync.dma_start(wp_sb[:], wp[:])

        # ---------- KV phase: q(ch0) + k/v projections ----------
        def emit_qp(ch, g):
            qp = ps.tile([P, NC2], F32, tag="mm", bufs=2, name=f"qp{ch}_{g}")
            for i in range(CT):
                nc.tensor.matmul(qp[:], wq_sb[:, i * C + g * P:i * C + (g + 1) * P],
                                 xTc_sb[ch][:, i * NC2:(i + 1) * NC2],
                                 start=(i == 0), stop=(i == CT - 1))
            qt = qtp.tile([P, NC2], BF16, tag=f"qT{g}", name=f"qt{ch}_{g}")
            nc.vector.tensor_copy(qt[:], qp[:])
            return qt

        # q for chunk 0 (fills PE while wk/wv transfers land)
        qT = [emit_qp(0, g) for g in range(G)]

        # kT: per head pair g -> [128, NKV] (rows 0-63 head 2g, 64-127 head 2g+1)
        kT_sb = const.tile([P, G * NKV], BF16, tag="kT")
        for g in range(G):
            kp = ps.tile([P, NKV], F32, tag="mm", bufs=2)
            for i in range(CT):
                nc.tensor.matmul(kp[:], wk_sb[:, i * C + g * P:i * C + (g + 1) * P],
                                 xkvT3[:, i, :], start=(i == 0), stop=(i == CT - 1))
            nc.scalar.activation(kT_sb[:, g * NKV:(g + 1) * NKV], kp[:], Copy)

        # v (+ ones col): per key chunk [128 keys, H*(HD+1)], col HD of each
        # head block stays 1.0 from the memset pre-fill
        vaug_sb = []
        for k in range(MCH):
            va = const.tile([P, H * (HD + 1)], BF16, tag=f"vaug{k}")
            nc.gpsimd.memset(va[:], 1.0)
            va3 = va[:].rearrange("p (h x) -> p h x", x=HD + 1)
            for half in range(2):
                vp = ps.tile([P, CT * HD], F32, tag="mm", bufs=2)
                for i in range(CT):
                    nc.tensor.matmul(vp[:], xkvT3[:, i, k * P:(k + 1) * P],
                                     wv_sb[:, i * C + half * CT * HD:
                                           i * C + (half + 1) * CT * HD],
                                     start=(i == 0), stop=(i == CT - 1))
                nc.scalar.activation(va3[:, CT * half:CT * half + CT, 0:HD],
                                     vp[:].rearrange("p (h x) -> p h x", x=HD), Copy)
            vaug_sb.append(va)

        # ---------- main loop over token chunks ----------
        def emit_proj(attn, ch):
            # output projection + bias (bias-add on Act, store on SP)
            for j in range(CT):
                pp = ps.tile([P, NC2], F32, tag="mm", bufs=2, name=f"pp{ch}_{j}")
                for i in range(CT):
                    nc.tensor.matmul(
                        pp[:], wp_sb[:, i * C + j * P:i * C + (j + 1) * P],
                        attn[i][:], start=(i == 0), stop=(i == CT - 1))
                oj = ojp.tile([P, NC2], BF16, tag="oj", name=f"oj{ch}_{j}")
                nc.scalar.activation(oj[:], pp[:],
                                     mybir.ActivationFunctionType.Identity,
                                     bias=pb_sb[:, j:j + 1])
                nc.sync.dma_start(out[:, j, ch, :], oj[:])

        pend_proj = None
        for ch in range(NCH):
            attn = []
            qT_next = []
            for g in range(G):
                at = att.tile([P, NC2], BF16, tag=f"attn{g}")
                for par in range(2):
                    h = 2 * g + par
                    base = par * HD
                    pts = []
                    for half in range(2):
                        st2 = ps.tile([P, 2 * NC2], F32, tag="st", bufs=2)
                        for hh in range(2):
                            k = 2 * half + hh
                            nc.tensor.matmul(
                                st2[:, hh * NC2:(hh + 1) * NC2],
                                kT_sb[base:base + HD,
                                      g * NKV + k * P:g * NKV + (k + 1) * P],
                                qT[g][base:base + HD, :],
                                start=True, stop=True)
                        pt = ptp.tile([P, 2 * NC2], BF16, tag="pt")
                        nc.scalar.activation(pt[:], st2[:], Exp, scale=SCALE)
                        pts.append(pt)
                    # previous chunk's proj runs here, overlapping this
                    # chunk's first exp latencies
                    if g == 0 and par == 1 and pend_proj is not None:
                        emit_proj(*pend_proj)
                        pend_proj = None
                    # overlap next chunk's q projection with the exp latency
                    if par == 0 and ch + 1 < NCH:
                        qT_next.append(emit_qp(ch + 1, g))
                    ov = ps.tile([HD + 1, NC2], F32, tag="ov", bufs=2)
                    for half in range(2):
                        for hh in range(2):
                            k = 2 * half + hh
                            nc.tensor.matmul(
                                ov[:], vaug_sb[k][:, h * (HD + 1):(h + 1) * (HD + 1)],
                                pts[half][:, hh * NC2:(hh + 1) * NC2],
                                start=(k == 0), stop=(k == MCH - 1))
                    rc = rcp.tile([1, NC2], F32, tag="rc")
                    nc.vector.reciprocal(rc[:], ov[HD:HD + 1, :])
                    rb = rcp.tile([HD, NC2], F32, tag="rb")
                    nc.gpsimd.partition_broadcast(rb[:], rc[:])
                    nc.vector.tensor_mul(at[base:base + HD, :], ov[0:HD, :], rb[:])
                attn.append(at)
            pend_proj = (attn, ch)
            qT = qT_next
        emit_proj(*pend_proj)

    nc.compile()
    return nc


def _get_nc():
    if "nc" not in _CACHE:
        _CACHE["nc"] = _build()
    return _CACHE["nc"]


def _prep_core_inputs(x, key_ind, q_w, kv_w, proj_w, proj_b):
    """Build the 8 per-core input maps (everything bf16 except idx/pb)."""
    bf16 = ml_dtypes.bfloat16

    def wT_pack(w):
        # [C(out), C(in)] weight -> transposed blocks [P, CT*C] bf16
        return np.ascontiguousarray(
            w.T.astype(np.float32).reshape(CT, P, C).transpose(1, 0, 2)
            .reshape(P, CT * C)).astype(bf16)

    wq = wT_pack(q_w)
    kvwT3 = kv_w.T.astype(np.float32).reshape(C, H, 2 * HD)
    wk = wT_pack(np.ascontiguousarray(kvwT3[:, :, :HD].reshape(C, C)).T)
    wv = wT_pack(np.ascontiguousarray(kvwT3[:, :, HD:].reshape(C, C)).T)
    wp = wT_pack(proj_w)
    pbp = np.ascontiguousarray(proj_b.astype(np.float32).reshape(CT, P).T)
    x = np.asarray(x, dtype=np.float32)
    in_maps = []
    for b in range(B):
        xb = x[b]                                   # [N, C]
        xTb = np.ascontiguousarray(
            xb.T.reshape(CT, P, NCH, NC2).transpose(2, 1, 0, 3)
            .reshape(NCH, P, CT * NC2)).astype(bf16)
        # int16 indices, index j at [j % 16, j // 16], replicated to all 128
        # partitions (16-partition wrap; walrus reads its own replica)
        idxb = np.ascontiguousarray(np.tile(
            np.asarray(key_ind[b]).astype(np.int16).reshape(NKV // 16, 16).T, (8, 1)))
        in_maps.append({
            "xT": xTb, "xr": xb.astype(bf16), "idx": idxb,
            "wq": wq, "wk": wk, "wv": wv, "wp": wp, "pb": pbp,
        })
    return in_maps


def kernel(x, key_ind, q_w, kv_w, proj_w, proj_b, _trace=False, _results=None):
    from concourse.bass_utils import run_bass_kernel_spmd

    nc = _get_nc()
    in_maps = _prep_core_inputs(x, key_ind, q_w, kv_w, proj_w, proj_b)
    res = run_bass_kernel_spmd(nc, in_maps, core_ids=list(range(B)), trace=_trace)
    if _results is not None:
        _results.append(res)
    outp = np.empty((B, N, C), dtype=np.float32)
    for b in range(B):
        o = res.results[b]["out"].astype(np.float32)   # [P, CT, NCH, NC2]
        outp[b] = o.transpose(2, 3, 1, 0).reshape(N, C)
    return outp


# revision 17
# speedup vs baseline: 1.3258x; 1.0105x over previous
"""Sparse attention kernel for Trainium2 (8 NeuronCores, data-parallel over batch).

Reference computation (per batch row b):
    q    = x @ q_w.T                                  [N, C]
    xkv  = x[key_ind]                                 [NKV, C]
    kv   = xkv @ kv_w.T -> per-head k, v              [NKV, 2C]
    attn = softmax((q*scale) @ k.T) @ v               [N, C]
    out  = attn @ proj_w.T + proj_b                   [N, C]

Per-core layout (core = one batch row), all SBUF data bf16, PSUM f32:
  - Everything transposed ("feature on partition"): qT [C, N] per head pair,
    kT [pair, NKV], scores ST [keys, tokens] so softmax needs no transposes;
    denominator via an all-ones pre-fill of the augmented v (65th column).
  - One PSUM pool, 8 banks: st 2x[128,1024] (two score tiles share one exp),
    ov 2x[128,512], mm 2x[128,512] (kv/q/proj accumulators share a tag).
  - KV gather via indirect SWDGE DMA + PE transposes (bf16: 1 cyc/row).
  - Engine balance: exp on Act, kv-phase evacs on Act, qt/recip/at-mul on
    DVE, broadcast + bias-add on Pool, loads/stores on SP — PE never waits.
"""
import os
import sys

sys.path.insert(0, "/opt/trn_rl_repo")

import numpy as np  # noqa: E402
import ml_dtypes  # noqa: E402

B, N, C = 8, 2048, 768
NKV = 512
H = 12
HD = C // H          # 64
SCALE = HD ** -0.5
P = 128
CT = C // P          # 6 feature tiles
NC2 = 512            # token chunk
NCH = N // NC2       # 4 chunks
MCH = NKV // P       # 4 key chunks
G = H // 2           # 6 head pairs

_CACHE = {}


def _build():
    import concourse.bass as bass
    import concourse.mybir as mybir
    import concourse.tile as tile
    from concourse import bacc
    from concourse.masks import make_identity
    from contextlib import ExitStack

    F32 = mybir.dt.float32
    BF16 = mybir.dt.bfloat16
    I16 = mybir.dt.int16
    Exp = mybir.ActivationFunctionType.Exp
    Copy = mybir.ActivationFunctionType.Copy

    nc = bacc.Bacc("TRN2", target_bir_lowering=False, debug=False, num_devices=8)

    xT = nc.dram_tensor("xT", [NCH, P, CT * NC2], BF16, kind="ExternalInput")
    xr = nc.dram_tensor("xr", [N, C], BF16, kind="ExternalInput")
    idx = nc.dram_tensor("idx", [P, NKV // 16], I16, kind="ExternalInput")
    wq = nc.dram_tensor("wq", [G, P, CT * P], BF16, kind="ExternalInput")
    wk = nc.dram_tensor("wk", [P, CT * C], BF16, kind="ExternalInput")
    wv = nc.dram_tensor("wv", [P, CT * C], BF16, kind="ExternalInput")
    wp = nc.dram_tensor("wp", [P, CT * C], BF16, kind="ExternalInput")
    pb = nc.dram_tensor("pb", [P, CT], F32, kind="ExternalInput")
    out = nc.dram_tensor("out", [P, CT, NCH, NC2], BF16, kind="ExternalOutput")

    with tile.TileContext(nc) as tc, ExitStack() as top:
        const = top.enter_context(tc.tile_pool(name="const", bufs=1))
        qtp = top.enter_context(tc.tile_pool(name="qtp", bufs=2))
        ptp = top.enter_context(tc.tile_pool(name="ptp", bufs=6))
        att = top.enter_context(tc.tile_pool(name="att", bufs=2))
        rcp = top.enter_context(tc.tile_pool(name="rcp", bufs=4))
        ojp = top.enter_context(tc.tile_pool(name="ojp", bufs=3))
        ps = top.enter_context(tc.tile_pool(name="ps", bufs=1, space="PSUM"))

        # ---------- input DMAs (order = DMA-engine priority) ----------
        pb_sb = const.tile([P, CT], F32, tag="pb")
        nc.sync.dma_start(pb_sb[:], pb[:])
        xTc_sb = []
        for ch in range(NCH):
            xTc_sb.append(const.tile([P, CT * NC2], BF16, tag=f"xT{ch}",
                                     name=f"xTc{ch}"))
        nc.sync.dma_start(xTc_sb[0][:], xT[0, :, :])
        idx_sb = const.tile([P, NKV // 16], I16, tag="idx")
        nc.sync.dma_start(idx_sb[:], idx[:])
        wq_sb = []
        for g in range(G):
            t = const.tile([P, CT * P], BF16, tag=f"wq{g}", name=f"wq{g}")
            nc.sync.dma_start(t[:], wq[g, :, :])
            wq_sb.append(t)

        # vaug ones pre-fill early (Pool is idle; must precede the v copies)
        vaug_sb = []
        for k in range(MCH):
            va = const.tile([P, H * (HD + 1)], BF16, tag=f"vaug{k}",
                            name=f"vaug{k}")
            nc.gpsimd.memset(va[:], 1.0)
            vaug_sb.append(va)

        # transposing gather: xkvT3[p, i, j] = xr[key_ind[j], i*128 + p]
        xkvT = const.tile([P, CT * NKV], BF16, tag="xkvT")
        xkvT3 = xkvT[:].rearrange("p (i m) -> p i m", i=CT)
        nc.gpsimd.dma_gather(
            out_ap=xkvT3, in_ap=xr[:], idxs_ap=idx_sb[:], num_idxs=NKV,
            num_idxs_reg=NKV, elem_size=C, transpose=True)

        wk_sb = const.tile([P, CT * C], BF16, tag="wk")
        nc.sync.dma_start(wk_sb[:], wk[:])
        wv_sb = const.tile([P, CT * C], BF16, tag="wv")
        nc.sync.dma_start(wv_sb[:], wv[:])
        for ch in range(1, NCH):
            nc.sync.dma_start(xTc_sb[ch][:], xT[ch, :, :])
        wp_sb = const.tile([P, CT * C], BF16, tag="wp")
        nc.sync.dma_start(wp_sb[:], wp[:])

        # ---------- KV phase: q(ch0) + k/v projections ----------
        def emit_qp(ch, g):
            qp = ps.tile([P, NC2], F32, tag="mm", bufs=2, name=f"qp{ch}_{g}")
            for i in range(CT):
                nc.tensor.matmul(qp[:], wq_sb[g][:, i * P:(i + 1) * P],
                                 xTc_sb[ch][:, i * NC2:(i + 1) * NC2],
                                 start=(i == 0), stop=(i == CT - 1))
            qt = qtp.tile([P, NC2], BF16, tag=f"qT{g}", name=f"qt{ch}_{g}")
            nc.vector.tensor_copy(qt[:], qp[:])
            return qt

        # kT per head pair g -> [128, NKV] (rows 0-63 head 2g, 64-127 head 2g+1)
        kT_sb = const.tile([P, G * NKV], BF16, tag="kT")

        def emit_kt(g):
            kp = ps.tile([P, NKV], F32, tag="mm", bufs=2, name=f"kp{g}")
            for i in range(CT):
                nc.tensor.matmul(kp[:], wk_sb[:, i * C + g * P:i * C + (g + 1) * P],
                                 xkvT3[:, i, :], start=(i == 0), stop=(i == CT - 1))
            nc.scalar.activation(kT_sb[:, g * NKV:(g + 1) * NKV], kp[:], Copy)

        def emit_vaug(k):
            # v (+ ones col): [128 keys, H*(HD+1)], col HD of each head block
            # stays 1.0 from the memset pre-fill
            va3 = vaug_sb[k][:].rearrange("p (h x) -> p h x", x=HD + 1)
            for half in range(2):
                vp = ps.tile([P, CT * HD], F32, tag="mm", bufs=2,
                             name=f"vp{k}_{half}")
                for i in range(CT):
                    nc.tensor.matmul(vp[:], xkvT3[:, i, k * P:(k + 1) * P],
                                     wv_sb[:, i * C + half * CT * HD:
                                           i * C + (half + 1) * CT * HD],
                                     start=(i == 0), stop=(i == CT - 1))
                nc.scalar.activation(va3[:, CT * half:CT * half + CT, 0:HD],
                                     vp[:].rearrange("p (h x) -> p h x", x=HD), Copy)

        # q for chunk 0 (fills PE while wk/wv/gather land), then the first two
        # kT pairs and all of vaug; kT(2..5) interleave into chunk 0's loop
        qT = [emit_qp(0, g) for g in range(G)]
        emit_kt(0)
        emit_kt(1)
        for k in range(MCH):
            emit_vaug(k)

        # ---------- main loop over token chunks ----------
        def emit_proj(attn, ch):
            # output projection + bias (bias-add on Act, store on SP)
            for j in range(CT):
                pp = ps.tile([P, NC2], F32, tag="mm", bufs=2, name=f"pp{ch}_{j}")
                for i in range(CT):
                    nc.tensor.matmul(
                        pp[:], wp_sb[:, i * C + j * P:i * C + (j + 1) * P],
                        attn[i][:], start=(i == 0), stop=(i == CT - 1))
                oj = ojp.tile([P, NC2], BF16, tag="oj", name=f"oj{ch}_{j}")
                nc.scalar.activation(oj[:], pp[:],
                                     mybir.ActivationFunctionType.Identity,
                                     bias=pb_sb[:, j:j + 1])
                nc.sync.dma_start(out[:, j, ch, :], oj[:])

        pend_proj = None
        for ch in range(NCH):
            attn = []
            qT_next = []
            for g in range(G):
                at = att.tile([P, NC2], BF16, tag=f"attn{g}")
                for par in range(2):
                    h = 2 * g + par
                    base = par * HD
                    pts = []
                    for half in range(2):
                        st2 = ps.tile([P, 2 * NC2], F32, tag="st", bufs=2)
                        for hh in range(2):
                            k = 2 * half + hh
                            nc.tensor.matmul(
                                st2[:, hh * NC2:(hh + 1) * NC2],
                                kT_sb[base:base + HD,
                                      g * NKV + k * P:g * NKV + (k + 1) * P],
                                qT[g][base:base + HD, :],
                                start=True, stop=True)
                        pt = ptp.tile([P, 2 * NC2], BF16, tag="pt")
                        nc.scalar.activation(pt[:], st2[:], Exp, scale=SCALE)
                        pts.append(pt)
                    # previous chunk's proj runs here, overlapping this
                    # chunk's first exp latencies
                    if g == 0 and par == 1 and pend_proj is not None:
                        emit_proj(*pend_proj)
                        pend_proj = None
                    # deferred kT pairs (chunk 0) and the next chunk's q
                    # projection overlap the exp latency
                    if par == 0 and ch == 0 and g + 2 < G:
                        emit_kt(g + 2)
                    if par == 0 and ch + 1 < NCH:
                        qT_next.append(emit_qp(ch + 1, g))
                    ov = ps.tile([HD + 1, NC2], F32, tag="ov", bufs=2)
                    for half in range(2):
                        for hh in range(2):
                            k = 2 * half + hh
                            nc.tensor.matmul(
                                ov[:], vaug_sb[k][:, h * (HD + 1):(h + 1) * (HD + 1)],
                                pts[half][:, hh * NC2:(hh + 1) * NC2],
                                start=(k == 0), stop=(k == MCH - 1))
                    rc = rcp.tile([1, NC2], F32, tag="rc")
                    nc.vector.reciprocal(rc[:], ov[HD:HD + 1, :])
                    rb = rcp.tile([HD, NC2], F32, tag="rb")
                    nc.gpsimd.partition_broadcast(rb[:], rc[:])
                    nc.vector.tensor_mul(at[base:base + HD, :], ov[0:HD, :], rb[:])
                attn.append(at)
            pend_proj = (attn, ch)
            qT = qT_next
        emit_proj(*pend_proj)

    nc.compile()
    return nc


def _get_nc():
    if "nc" not in _CACHE:
        _CACHE["nc"] = _build()
    return _CACHE["nc"]


def _prep_core_inputs(x, key_ind, q_w, kv_w, proj_w, proj_b):
    """Build the 8 per-core input maps (everything bf16 except idx/pb)."""
    bf16 = ml_dtypes.bfloat16

    def wT_pack(w):
        # [C(out), C(in)] weight -> transposed blocks [P, CT*C] bf16
        return np.ascontiguousarray(
            w.T.astype(np.float32).reshape(CT, P, C).transpose(1, 0, 2)
            .reshape(P, CT * C)).astype(bf16)

    # wq repacked per head pair: [G, P, CT*128]
    wq = np.ascontiguousarray(
        wT_pack(q_w).reshape(P, CT, G, P).transpose(2, 0, 1, 3)
        .reshape(G, P, CT * P))
    kvwT3 = kv_w.T.astype(np.float32).reshape(C, H, 2 * HD)
    wk = wT_pack(np.ascontiguousarray(kvwT3[:, :, :HD].reshape(C, C)).T)
    wv = wT_pack(np.ascontiguousarray(kvwT3[:, :, HD:].reshape(C, C)).T)
    wp = wT_pack(proj_w)
    pbp = np.ascontiguousarray(proj_b.astype(np.float32).reshape(CT, P).T)
    x = np.asarray(x, dtype=np.float32)
    in_maps = []
    for b in range(B):
        xb = x[b]                                   # [N, C]
        xTb = np.ascontiguousarray(
            xb.T.reshape(CT, P, NCH, NC2).transpose(2, 1, 0, 3)
            .reshape(NCH, P, CT * NC2)).astype(bf16)
        # int16 indices, index j at [j % 16, j // 16], replicated to all 128
        # partitions (16-partition wrap; walrus reads its own replica)
        idxb = np.ascontiguousarray(np.tile(
            np.asarray(key_ind[b]).astype(np.int16).reshape(NKV // 16, 16).T, (8, 1)))
        in_maps.append({
            "xT": xTb, "xr": xb.astype(bf16), "idx": idxb,
            "wq": wq, "wk": wk, "wv": wv, "wp": wp, "pb": pbp,
        })
    return in_maps


def kernel(x, key_ind, q_w, kv_w, proj_w, proj_b, _trace=False, _results=None):
    from concourse.bass_utils import run_bass_kernel_spmd

    nc = _get_nc()
    in_maps = _prep_core_inputs(x, key_ind, q_w, kv_w, proj_w, proj_b)
    res = run_bass_kernel_spmd(nc, in_maps, core_ids=list(range(B)), trace=_trace)
    if _results is not None:
        _results.append(res)
    outp = np.empty((B, N, C), dtype=np.float32)
    for b in range(B):
        o = res.results[b]["out"].astype(np.float32)   # [P, CT, NCH, NC2]
        outp[b] = o.transpose(2, 3, 1, 0).reshape(N, C)
    return outp


# revision 22
# speedup vs baseline: 1.3413x; 1.0118x over previous
"""Sparse attention kernel for Trainium2 (8 NeuronCores, data-parallel over batch).

Reference computation (per batch row b):
    q    = x @ q_w.T                                  [N, C]
    xkv  = x[key_ind]                                 [NKV, C]
    kv   = xkv @ kv_w.T -> per-head k, v              [NKV, 2C]
    attn = softmax((q*scale) @ k.T) @ v               [N, C]
    out  = attn @ proj_w.T + proj_b                   [N, C]

Per-core layout (core = one batch row), all SBUF data bf16, PSUM f32:
  - Everything transposed ("feature on partition"): qT [C, N] per head pair,
    kT [pair, NKV], scores ST [keys, tokens] so softmax needs no transposes;
    denominator via an all-ones pre-fill of the augmented v (65th column).
  - One PSUM pool, 8 banks: st 2x[128,1024] (two score tiles share one exp),
    ov 2x[128,512], mm 2x[128,512] (kv/q/proj accumulators share a tag).
  - KV gather via indirect SWDGE DMA + PE transposes (bf16: 1 cyc/row).
  - Engine balance: exp on Act, kv-phase evacs on Act, qt/recip/at-mul on
    DVE, broadcast + bias-add on Pool, loads/stores on SP — PE never waits.
"""
import os
import sys

sys.path.insert(0, "/opt/trn_rl_repo")

import numpy as np  # noqa: E402
import ml_dtypes  # noqa: E402

B, N, C = 8, 2048, 768
NKV = 512
H = 12
HD = C // H          # 64
SCALE = HD ** -0.5
P = 128
CT = C // P          # 6 feature tiles
NC2 = 512            # token chunk
NCH = N // NC2       # 4 chunks
MCH = NKV // P       # 4 key chunks
G = H // 2           # 6 head pairs

_CACHE = {}


def _build():
    import concourse.bass as bass
    import concourse.mybir as mybir
    import concourse.tile as tile
    from concourse import bacc
    from concourse.masks import make_identity
    from contextlib import ExitStack

    F32 = mybir.dt.float32
    BF16 = mybir.dt.bfloat16
    I16 = mybir.dt.int16
    Exp = mybir.ActivationFunctionType.Exp
    Copy = mybir.ActivationFunctionType.Copy

    nc = bacc.Bacc("TRN2", target_bir_lowering=False, debug=False, num_devices=8)

    xT = nc.dram_tensor("xT", [NCH, P, CT * NC2], BF16, kind="ExternalInput")
    xr = nc.dram_tensor("xr", [N, C], BF16, kind="ExternalInput")
    idx = nc.dram_tensor("idx", [P, NKV // 16], I16, kind="ExternalInput")
    wq = nc.dram_tensor("wq", [G, P, CT * P], BF16, kind="ExternalInput")
    wk = nc.dram_tensor("wk", [P, CT * C], BF16, kind="ExternalInput")
    wv = nc.dram_tensor("wv", [P, CT * C], BF16, kind="ExternalInput")
    wp = nc.dram_tensor("wp", [P, CT * C], BF16, kind="ExternalInput")
    pb = nc.dram_tensor("pb", [P, CT], F32, kind="ExternalInput")
    out = nc.dram_tensor("out", [P, CT, NCH, NC2], BF16, kind="ExternalOutput")

    with tile.TileContext(nc) as tc, ExitStack() as top:
        const = top.enter_context(tc.tile_pool(name="const", bufs=1))
        qtp = top.enter_context(tc.tile_pool(name="qtp", bufs=2))
        ptp = top.enter_context(tc.tile_pool(name="ptp", bufs=6))
        att = top.enter_context(tc.tile_pool(name="att", bufs=2))
        rcp = top.enter_context(tc.tile_pool(name="rcp", bufs=4))
        ojp = top.enter_context(tc.tile_pool(name="ojp", bufs=3))
        ps = top.enter_context(tc.tile_pool(name="ps", bufs=1, space="PSUM"))

        # ---------- input DMAs (order = DMA-engine priority) ----------
        pb_sb = const.tile([P, CT], F32, tag="pb")
        nc.sync.dma_start(pb_sb[:], pb[:])
        xTc_sb = []
        for ch in range(NCH):
            xTc_sb.append(const.tile([P, CT * NC2], BF16, tag=f"xT{ch}",
                                     name=f"xTc{ch}"))
        nc.sync.dma_start(xTc_sb[0][:], xT[0, :, :])
        idx_sb = const.tile([P, NKV // 16], I16, tag="idx")
        nc.sync.dma_start(idx_sb[:], idx[:])
        wq_sb = []
        for g in range(G):
            t = const.tile([P, CT * P], BF16, tag=f"wq{g}", name=f"wq{g}")
            nc.sync.dma_start(t[:], wq[g, :, :])
            wq_sb.append(t)

        # vaug ones pre-fill early (Pool is idle; must precede the v copies)
        vaug_sb = []
        for k in range(MCH):
            va = const.tile([P, H * (HD + 1)], BF16, tag=f"vaug{k}",
                            name=f"vaug{k}")
            nc.gpsimd.memset(va[:], 1.0)
            vaug_sb.append(va)

        # transposing gather: xkvT3[p, i, j] = xr[key_ind[j], i*128 + p]
        xkvT = const.tile([P, CT * NKV], BF16, tag="xkvT")
        xkvT3 = xkvT[:].rearrange("p (i m) -> p i m", i=CT)
        nc.gpsimd.dma_gather(
            out_ap=xkvT3, in_ap=xr[:], idxs_ap=idx_sb[:], num_idxs=NKV,
            num_idxs_reg=NKV, elem_size=C, transpose=True)

        wk_sb = const.tile([P, CT * C], BF16, tag="wk")
        nc.sync.dma_start(wk_sb[:], wk[:])
        wv_sb = const.tile([P, CT * C], BF16, tag="wv")
        nc.sync.dma_start(wv_sb[:], wv[:])
        for ch in range(1, NCH):
            nc.sync.dma_start(xTc_sb[ch][:], xT[ch, :, :])
        wp_sb = const.tile([P, CT * C], BF16, tag="wp")
        nc.sync.dma_start(wp_sb[:], wp[:])

        # ---------- KV phase: q(ch0) + k/v projections ----------
        def emit_qp(ch, g):
            qp = ps.tile([P, NC2], F32, tag="mm", bufs=2, name=f"qp{ch}_{g}")
            for i in range(CT):
                nc.tensor.matmul(qp[:], wq_sb[g][:, i * P:(i + 1) * P],
                                 xTc_sb[ch][:, i * NC2:(i + 1) * NC2],
                                 start=(i == 0), stop=(i == CT - 1))
            qt = qtp.tile([P, NC2], BF16, tag=f"qT{g}", name=f"qt{ch}_{g}")
            nc.vector.tensor_copy(qt[:], qp[:])
            return qt

        # kT per head pair g -> [128, NKV] (rows 0-63 head 2g, 64-127 head 2g+1)
        kT_sb = const.tile([P, G * NKV], BF16, tag="kT")

        def emit_kt(g):
            kp = ps.tile([P, NKV], F32, tag="mm", bufs=2, name=f"kp{g}")
            for i in range(CT):
                nc.tensor.matmul(kp[:], wk_sb[:, i * C + g * P:i * C + (g + 1) * P],
                                 xkvT3[:, i, :], start=(i == 0), stop=(i == CT - 1))
            nc.scalar.activation(kT_sb[:, g * NKV:(g + 1) * NKV], kp[:], Copy)

        def emit_vaug(k):
            # v (+ ones col): [128 keys, H*(HD+1)], col HD of each head block
            # stays 1.0 from the memset pre-fill
            va3 = vaug_sb[k][:].rearrange("p (h x) -> p h x", x=HD + 1)
            for half in range(2):
                vp = ps.tile([P, CT * HD], F32, tag="mm", bufs=2,
                             name=f"vp{k}_{half}")
                for i in range(CT):
                    nc.tensor.matmul(vp[:], xkvT3[:, i, k * P:(k + 1) * P],
                                     wv_sb[:, i * C + half * CT * HD:
                                           i * C + (half + 1) * CT * HD],
                                     start=(i == 0), stop=(i == CT - 1))
                nc.scalar.activation(va3[:, CT * half:CT * half + CT, 0:HD],
                                     vp[:].rearrange("p (h x) -> p h x", x=HD), Copy)

        # q for chunk 0 (fills PE while wk/wv/gather land), then the first two
        # kT pairs and all of vaug; kT(2..5) interleave into chunk 0's loop
        qT = [emit_qp(0, g) for g in range(G)]
        emit_kt(0)
        emit_kt(1)
        for k in range(MCH):
            emit_vaug(k)

        # ---------- main loop over token chunks ----------
        def emit_oj(pp, ch, j):
            # bias-add alternates Act/DVE to balance engine load; store on SP
            oj = ojp.tile([P, NC2], BF16, tag="oj", name=f"oj{ch}_{j}")
            if j % 2 == 0:
                nc.scalar.activation(oj[:], pp[:],
                                     mybir.ActivationFunctionType.Identity,
                                     bias=pb_sb[:, j:j + 1])
            else:
                nc.vector.tensor_scalar_add(oj[:], pp[:], pb_sb[:, j:j + 1])
            nc.sync.dma_start(out[:, j, ch, :], oj[:])

        def emit_proj(attn, ch):
            for j in range(CT):
                pp = ps.tile([P, NC2], F32, tag="mm", bufs=2, name=f"pp{ch}_{j}")
                for i in range(CT):
                    nc.tensor.matmul(
                        pp[:], wp_sb[:, i * C + j * P:i * C + (j + 1) * P],
                        attn[i][:], start=(i == 0), stop=(i == CT - 1))
                emit_oj(pp, ch, j)

        def wp_step(pp, i, j, start):
            nc.tensor.matmul(
                pp[:], wp_sb[:, i * C + j * P:i * C + (j + 1) * P],
                attn[i][:], start=start, stop=False, skip_group_check=True)

        pend_proj = None
        for ch in range(NCH):
            attn = []
            qT_next = []
            partial = {}       # last chunk: j -> incremental proj accumulator
            for g in range(G):
                at = att.tile([P, NC2], BF16, tag=f"attn{g}")
                for par in range(2):
                    h = 2 * g + par
                    base = par * HD
                    pts = []
                    for half in range(2):
                        st2 = ps.tile([P, 2 * NC2], F32, tag="st", bufs=2)
                        for hh in range(2):
                            k = 2 * half + hh
                            nc.tensor.matmul(
                                st2[:, hh * NC2:(hh + 1) * NC2],
                                kT_sb[base:base + HD,
                                      g * NKV + k * P:g * NKV + (k + 1) * P],
                                qT[g][base:base + HD, :],
                                start=True, stop=True)
                        pt = ptp.tile([P, 2 * NC2], BF16, tag="pt")
                        nc.scalar.activation(pt[:], st2[:], Exp, scale=SCALE)
                        pts.append(pt)
                    # previous chunk's proj runs here, overlapping this
                    # chunk's first exp latencies
                    if g == 0 and par == 1 and pend_proj is not None:
                        emit_proj(*pend_proj)
                        pend_proj = None
                    # deferred kT pairs (chunk 0) and the next chunk's q
                    # projection overlap the exp latency
                    if par == 0 and ch == 0 and g + 2 < G:
                        emit_kt(g + 2)
                    if par == 0 and ch + 1 < NCH:
                        qT_next.append(emit_qp(ch + 1, g))
                    if par == 0 and ch + 1 == NCH and g >= 2:
                        # last chunk: start two incremental proj accumulators
                        # in the mm slots the q-interleave vacated
                        for j in range(min(g - 1, 2)):
                            if j not in partial:
                                pp = ps.tile([P, NC2], F32, tag="mm", bufs=2,
                                             name=f"ppl{j}")
                                wp_step(pp, 0, j, True)
                                partial[j] = (pp, 1)
                            pp, ni = partial[j]
                            while ni < g:
                                wp_step(pp, ni, j, False)
                                ni += 1
                            partial[j] = (pp, ni)
                    ov = ps.tile([HD + 1, NC2], F32, tag="ov", bufs=2)
                    for half in range(2):
                        for hh in range(2):
                            k = 2 * half + hh
                            nc.tensor.matmul(
                                ov[:], vaug_sb[k][:, h * (HD + 1):(h + 1) * (HD + 1)],
                                pts[half][:, hh * NC2:(hh + 1) * NC2],
                                start=(k == 0), stop=(k == MCH - 1))
                    rc = rcp.tile([1, NC2], F32, tag="rc")
                    nc.vector.reciprocal(rc[:], ov[HD:HD + 1, :])
                    rb = rcp.tile([HD, NC2], F32, tag="rb")
                    nc.gpsimd.partition_broadcast(rb[:], rc[:])
                    nc.vector.tensor_mul(at[base:base + HD, :], ov[0:HD, :], rb[:])
                attn.append(at)
            if ch + 1 < NCH:
                pend_proj = (attn, ch)
                qT = qT_next
            else:
                # last chunk: four more accumulators in the freed st halves,
                # catch everyone up through head pair 4, then the finals
                st_a = ps.tile([P, 2 * NC2], F32, tag="st", bufs=2, name="ppl_a")
                st_b = ps.tile([P, 2 * NC2], F32, tag="st", bufs=2, name="ppl_b")
                for j in range(2, CT):
                    half = (j - 2) % 2
                    src = st_a if j < 4 else st_b
                    pp = src[:, half * NC2:(half + 1) * NC2]
                    wp_step(pp, 0, j, True)
                    partial[j] = (pp, 1)
                for j in range(CT):
                    pp, ni = partial[j]
                    while ni < G - 1:
                        wp_step(pp, ni, j, False)
                        ni += 1
                    partial[j] = (pp, ni)
                for j in range(CT):
                    pp, ni = partial[j]
                    assert ni == G - 1
                    nc.tensor.matmul(
                        pp[:], wp_sb[:, 5 * C + j * P:5 * C + (j + 1) * P],
                        attn[5][:], start=False, stop=True, skip_group_check=True)
                    emit_oj(pp, ch, j)

    nc.compile()
    return nc


def _get_nc():
    if "nc" not in _CACHE:
        _CACHE["nc"] = _build()
    return _CACHE["nc"]


def _prep_core_inputs(x, key_ind, q_w, kv_w, proj_w, proj_b):
    """Build the 8 per-core input maps (everything bf16 except idx/pb)."""
    bf16 = ml_dtypes.bfloat16

    def wT_pack(w):
        # [C(out), C(in)] weight -> transposed blocks [P, CT*C] bf16
        return np.ascontiguousarray(
            w.T.astype(np.float32).reshape(CT, P, C).transpose(1, 0, 2)
            .reshape(P, CT * C)).astype(bf16)

    # wq repacked per head pair: [G, P, CT*128]
    wq = np.ascontiguousarray(
        wT_pack(q_w).reshape(P, CT, G, P).transpose(2, 0, 1, 3)
        .reshape(G, P, CT * P))
    kvwT3 = kv_w.T.astype(np.float32).reshape(C, H, 2 * HD)
    wk = wT_pack(np.ascontiguousarray(kvwT3[:, :, :HD].reshape(C, C)).T)
    wv = wT_pack(np.ascontiguousarray(kvwT3[:, :, HD:].reshape(C, C)).T)
    wp = wT_pack(proj_w)
    pbp = np.ascontiguousarray(proj_b.astype(np.float32).reshape(CT, P).T)
    x = np.asarray(x, dtype=np.float32)
    in_maps = []
    for b in range(B):
        xb = x[b]                                   # [N, C]
        xTb = np.ascontiguousarray(
            xb.T.reshape(CT, P, NCH, NC2).transpose(2, 1, 0, 3)
            .reshape(NCH, P, CT * NC2)).astype(bf16)
        # int16 indices, index j at [j % 16, j // 16], replicated to all 128
        # partitions (16-partition wrap; walrus reads its own replica)
        idxb = np.ascontiguousarray(np.tile(
            np.asarray(key_ind[b]).astype(np.int16).reshape(NKV // 16, 16).T, (8, 1)))
        in_maps.append({
            "xT": xTb, "xr": xb.astype(bf16), "idx": idxb,
            "wq": wq, "wk": wk, "wv": wv, "wp": wp, "pb": pbp,
        })
    return in_maps


def kernel(x, key_ind, q_w, kv_w, proj_w, proj_b, _trace=False, _results=None):
    from concourse.bass_utils import run_bass_kernel_spmd

    nc = _get_nc()
    in_maps = _prep_core_inputs(x, key_ind, q_w, kv_w, proj_w, proj_b)
    res = run_bass_kernel_spmd(nc, in_maps, core_ids=list(range(B)), trace=_trace)
    if _results is not None:
        _results.append(res)
    outp = np.empty((B, N, C), dtype=np.float32)
    for b in range(B):
        o = res.results[b]["out"].astype(np.float32)   # [P, CT, NCH, NC2]
        outp[b] = o.transpose(2, 3, 1, 0).reshape(N, C)
    return outp
